# revision 32
# baseline (speedup 1.0000x reference)
"""GQA attention (dense_transformer) on 8 TRN2 NeuronCores.

Sharding: core c = b*4 + j (b = batch 0..1, j = tensor-parallel rank 0..3)
computes q-heads 8j..8j+7 (kv heads 2j, 2j+1) of batch b.  The default
exchange is an 8-core AllToAll (ag_mode='a2a'): after each 512-token
attention window, core c ships each destination core its 512 attn^T
features for that core's 64-token slice; every core then applies the FULL
wo to its gathered 2048 features and emits final output columns for its
token slices of BOTH batches.  Host assembly stitches token slices.  The
A2A moves ~8x fewer bytes than gathering attn over rank groups, which
measured ~3x faster end-to-end on hardware.

Structure: window w-1's wo matmuls run as work units spread through
window w's per-head attention loop, so the PE does not sit behind the
collective; x chunks/rope tables prefetch one window ahead.  (Optional
flags also interleave window w+1's projections into window w and spread
the cc_in writes per head pair; measurements on this terminal were
inconclusive, so the simpler schedule ships.)  Causal diagonal tiles are subranged
plus one 128x128 triangle mask (on the otherwise idle Pool engine).
Inputs, rope and attention run in bf16 (measured 7.7e-3 rel err vs the
f32 reference, tolerance 2e-2); softmax statistics accumulate in f32
PSUM via an appended ones-column in the PV matmul.

Self-contained: hardcodes shapes from the problem spec.
"""
import os
import sys

sys.path.insert(0, "/opt/trn_rl_repo")

from contextlib import ExitStack

import numpy as np
import ml_dtypes

import concourse.bass as bass
import concourse.mybir as mybir
import concourse.tile as tile
from concourse import bacc
from concourse.bass_utils import run_bass_kernel_spmd
from concourse.masks import make_identity

HIDDEN = 2048
N_HEADS = 32
N_KV_HEADS = 8
HEAD_DIM = 64
B_FULL, T_FULL = 2, 2048

NCORES = 8
NTP = 4                       # tensor-parallel ranks per batch group
NHL = N_HEADS // NTP          # 8 local q heads
NKVL = N_KV_HEADS // NTP      # 2 local kv heads
QF = NHL * HEAD_DIM           # 512 local q features
KF = NKVL * HEAD_DIM          # 128 local kv features
COLS = HIDDEN // NTP          # 512 output columns per rank
TCP = 256                     # projection t-chunk width
TCA = 512                     # attention window width
P = 128

F32 = mybir.dt.float32
F32R = mybir.dt.float32r
BF16 = mybir.dt.bfloat16

SCALE = 1.0 / np.sqrt(HEAD_DIM)

LAST_EXEC_NS = None
LAST_RESULTS = None


def build_kernel(T=T_FULL, repeat=1, no_ag=False, ag_mode='full8',
                 shared_out=False, gsz=None, interleave=True, ccin_spread=True):
    """One SPMD program; every core runs the same code on its shard."""
    assert T % TCA == 0
    NW = T // TCA             # attention windows
    KH = HIDDEN // P          # 16 k-tiles over hidden
    NTT = T // P              # tk tiles total
    WTK = TCA // P            # tk tiles per window (4)

    nc = bacc.Bacc("TRN2", debug=False)

    xT = nc.dram_tensor("xT", [HIDDEN, T], BF16, kind="ExternalInput")
    wqT = nc.dram_tensor("wqT", [HIDDEN, QF], BF16, kind="ExternalInput")
    wkT = nc.dram_tensor("wkT", [HIDDEN, KF], BF16, kind="ExternalInput")
    wvT = nc.dram_tensor("wvT", [HIDDEN, KF], BF16, kind="ExternalInput")
    if ag_mode == 'a2a':
        woT = nc.dram_tensor("woT", [HIDDEN, HIDDEN], BF16, kind="ExternalInput")
    else:
        woT = nc.dram_tensor("woT", [2 * HIDDEN, COLS], BF16,
                             kind="ExternalInput")
    cosT = nc.dram_tensor("cosT", [P, T], BF16, kind="ExternalInput")
    sinTs = nc.dram_tensor("sinTs", [P, T], BF16, kind="ExternalInput")
    swp = nc.dram_tensor("swp", [P, P], BF16, kind="ExternalInput")
    msk = nc.dram_tensor("msk", [P, P], BF16, kind="ExternalInput")
    if ag_mode == 'a2a':
        out = nc.dram_tensor("out", [2 * HIDDEN, (T // TCA) * 64], F32,
                             kind="ExternalOutput")
    else:
        out = nc.dram_tensor("out", [COLS, T], F32, kind="ExternalOutput")

    n_gather = NCORES if ag_mode in ('full8', 'single8') else NTP
    ccspace = "Shared" if shared_out else "Local"
    if ag_mode == 'a2a':
        GSZ = gsz if gsz else 1
        NPAIR = NW // GSZ
        TSL = 64 * GSZ            # my token-slice width per exchange
        cc_in = [nc.dram_tensor(f"cc_in{i}", [NCORES * QF, TSL], BF16)
                 for i in range(NPAIR * repeat)]
        cc_out = [nc.dram_tensor(f"cc_out{i}", [NCORES * QF, TSL], BF16)
                  for i in range(NPAIR * repeat)]
    elif ag_mode == 'single8':
        cc_in = [nc.dram_tensor(f"cc_in{i}", [QF, T], BF16)
                 for i in range(repeat)]
        cc_out = [nc.dram_tensor(f"cc_out{i}", [n_gather * QF, T], BF16,
                                 addr_space=ccspace)
                  for i in range(repeat)]
    else:
        GSZ = gsz if gsz else (2 if NW % 2 == 0 else 1)
        NPAIR = NW // GSZ
        cc_in = [nc.dram_tensor(f"cc_in{i}", [QF, GSZ * TCA], BF16)
                 for i in range(NPAIR * repeat)]
        cc_out = [nc.dram_tensor(f"cc_out{i}", [n_gather * QF, GSZ * TCA], BF16,
                                 addr_space=ccspace)
                  for i in range(NPAIR * repeat)]
    groups = ([[0, 1, 2, 3, 4, 5, 6, 7]] if ag_mode in ('full8', 'a2a')
              else [[0, 1, 2, 3], [4, 5, 6, 7]])

    NKW = 2 * KH if ag_mode == 'full8' else KH

    with tile.TileContext(nc) as tc, ExitStack() as est:
        consts = est.enter_context(tc.tile_pool(name="consts", bufs=1))
        kpool = est.enter_context(tc.tile_pool(name="kpool", bufs=1))
        xcpool = est.enter_context(tc.tile_pool(name="xcpool", bufs=34))
        # (a2a mode carries a full-wo tile; SBUF is tight there)
        stream = est.enter_context(tc.tile_pool(name="stream", bufs=3))
        qrpool = est.enter_context(tc.tile_pool(name="qrpool", bufs=10))
        ppool = est.enter_context(tc.tile_pool(name="ppool", bufs=4))
        atpool = est.enter_context(tc.tile_pool(name="atpool", bufs=6))
        agpool = est.enter_context(tc.tile_pool(
            name="agpool", bufs=2 if ag_mode == 'a2a' else 32))
        small = est.enter_context(tc.tile_pool(name="small", bufs=2))
        ps_proj = est.enter_context(tc.tile_pool(name="ps_proj", bufs=2, space="PSUM"))
        ps_s = est.enter_context(tc.tile_pool(name="ps_s", bufs=2, space="PSUM"))
        ps_pv = est.enter_context(tc.tile_pool(name="ps_pv", bufs=2, space="PSUM"))
        ps_y = est.enter_context(tc.tile_pool(name="ps_y", bufs=1, space="PSUM"))
        ps_misc = est.enter_context(tc.tile_pool(name="ps_misc", bufs=1, space="PSUM"))

        # ---- constants (DMA order matters for startup: weights first, then
        # rope tables, mask, wo) ----
        swp_sb = consts.tile([P, P], BF16)
        wq_sb = consts.tile([P, KH, QF], BF16)
        wk_sb = consts.tile([P, KH, KF], BF16)
        wv_sb = consts.tile([P, KH, KF], BF16)
        WOF = HIDDEN if ag_mode == 'a2a' else COLS
        wo_sb = consts.tile([P, NKW, WOF], BF16)
        cos_sb = consts.tile([P, 2, TCA], BF16)
        sin_sb = consts.tile([P, 2, TCA], BF16)
        msk_sb = consts.tile([P, P], BF16)
        id_sb = consts.tile([P, P], BF16)
        id_f32 = consts.tile([P, P], F32)
        ones_sb = consts.tile([1, HEAD_DIM], F32R)
        ones_bf = consts.tile([P, 1], BF16)
        ones_f32 = consts.tile([P, 1], F32)
        ones_row_f32 = consts.tile([1, HEAD_DIM], F32)

        xv = xT[:, :].rearrange("(t p) n -> p t n", p=P)
        nc.sync.dma_start(out=swp_sb, in_=swp[:, :])
        wqv = wqT[:, :].rearrange("(t p) f -> p t f", p=P)
        wkv = wkT[:, :].rearrange("(t p) f -> p t f", p=P)
        wvv = wvT[:, :].rearrange("(t p) f -> p t f", p=P)
        wov = woT[:, :].rearrange("(t p) f -> p t f", p=P)
        xc0 = []
        for k in range(KH):
            nc.sync.dma_start(out=wq_sb[:, k, :], in_=wqv[:, k, :])
            xt_ = xcpool.tile([P, TCP], BF16, tag="xc")
            nc.sync.dma_start(out=xt_, in_=xv[:, k, 0:TCP])
            xc0.append(xt_)
        for k in range(KH):
            nc.sync.dma_start(out=wk_sb[:, k, :], in_=wkv[:, k, :])
            nc.sync.dma_start(out=wv_sb[:, k, :], in_=wvv[:, k, :])
        sl = slice(0, TCA)
        nc.sync.dma_start(out=cos_sb[:, 0, :], in_=cosT[:, sl])
        nc.sync.dma_start(out=sin_sb[:, 0, :], in_=sinTs[:, sl])
        nc.sync.dma_start(out=msk_sb, in_=msk[:, :])

        make_identity(nc, id_f32)
        nc.vector.tensor_copy(id_sb, id_f32)
        nc.vector.memset(ones_f32, 1.0)
        nc.vector.memset(ones_bf, 1.0)
        nc.vector.memset(ones_row_f32, 1.0)
        nc.vector.tensor_copy(ones_sb, ones_row_f32)

        # ---- persistent K / V accumulators ----
        KA = kpool.tile([P, T], BF16, tag="KA")   # [g0; g0] roped K^T
        KB = kpool.tile([P, T], BF16, tag="KB")   # [g1; g1]
        # V natural layout per tk-tile: cols = [V_g0 (64) | 1 | V_g1 (64) | 1]
        vaug = kpool.tile([P, NTT, 2 * HEAD_DIM + 2], BF16, tag="vaug")
        for t in range(NTT):
            nc.vector.tensor_copy(vaug[:, t, HEAD_DIM:HEAD_DIM + 1], ones_bf)
            nc.vector.tensor_copy(vaug[:, t, 2 * HEAD_DIM + 1:2 * HEAD_DIM + 2],
                                  ones_bf)


        def rope(raw_sb, cs, ss, out_ap):
            """out = raw*cos + swap(raw)*sin_signed  (all [P, TCP])."""
            sw_ps = ps_misc.tile([P, TCP], F32, tag="misc")
            nc.tensor.matmul(sw_ps, lhsT=swp_sb, rhs=raw_sb, start=True, stop=True)
            m2 = stream.tile([P, TCP], BF16, tag="tmp")
            nc.vector.tensor_tensor(out=m2, in0=sw_ps, in1=ss, op=mybir.AluOpType.mult)
            nc.vector.tensor_tensor(out=out_ap, in0=raw_sb, in1=cs,
                                    op=mybir.AluOpType.mult)
            nc.vector.tensor_tensor(out=out_ap, in0=out_ap, in1=m2,
                                    op=mybir.AluOpType.add)

        def load_chunk(c):
            csl = slice(c * TCP, (c + 1) * TCP)
            xc = []
            for k in range(KH):
                t_ = xcpool.tile([P, TCP], BF16, tag="xc")
                nc.sync.dma_start(out=t_, in_=xv[:, k, csl])
                xc.append(t_)
            return xc

        def _csin(c):
            slot = (c // 2) % 2
            lsl_c = slice((c % 2) * TCP, (c % 2 + 1) * TCP)
            return cos_sb[:, slot, lsl_c], sin_sb[:, slot, lsl_c]

        def project_q(c, qrope, half, xc):
            """Q projections + rope for t-chunk c; writes qrope[:][half]."""
            hsl = slice(half * TCP, (half + 1) * TCP)
            cs, ss = _csin(c)
            # Q: 4 head-pair tiles
            for m in range(4):
                q_ps = ps_proj.tile([P, TCP], F32, tag="proj")
                for k in range(KH):
                    nc.tensor.matmul(q_ps, lhsT=wq_sb[:, k, m * P:(m + 1) * P],
                                     rhs=xc[k], start=(k == 0), stop=(k == KH - 1))
                raw = stream.tile([P, TCP], BF16, tag="raw")
                nc.vector.tensor_copy(raw, q_ps)
                rope(raw, cs, ss, qrope[m][:, hsl])

        def project_kv(c, xc):
            """K/V projections + rope + layout for t-chunk c."""
            cs, ss = _csin(c)
            k_ps = ps_proj.tile([P, TCP], F32, tag="proj")
            for k in range(KH):
                nc.tensor.matmul(k_ps, lhsT=wk_sb[:, k, :], rhs=xc[k],
                                 start=(k == 0), stop=(k == KH - 1))
            kraw = stream.tile([P, TCP], BF16, tag="raw")
            nc.vector.tensor_copy(kraw, k_ps)
            krope = stream.tile([P, TCP], BF16, tag="raw")
            rope(kraw, cs, ss, krope)
            csl = slice(c * TCP, (c + 1) * TCP)
            nc.gpsimd.tensor_copy(KA[0:64, csl], krope[0:64, :])
            nc.gpsimd.tensor_copy(KA[64:128, csl], krope[0:64, :])
            nc.gpsimd.tensor_copy(KB[0:64, csl], krope[64:128, :])
            nc.gpsimd.tensor_copy(KB[64:128, csl], krope[64:128, :])
            # V (as V^T) then transpose into vaug
            v_ps = ps_proj.tile([P, TCP], F32, tag="proj")
            for k in range(KH):
                nc.tensor.matmul(v_ps, lhsT=wv_sb[:, k, :], rhs=xc[k],
                                 start=(k == 0), stop=(k == KH - 1))
            vt = stream.tile([P, TCP], BF16, tag="raw")
            nc.vector.tensor_copy(vt, v_ps)
            for tt in range(TCP // P):
                tp_ps = ps_misc.tile([P, P], BF16, tag="misc")
                nc.tensor.transpose(tp_ps, vt[:, tt * P:(tt + 1) * P], id_sb)
                tkt = c * (TCP // P) + tt
                nc.vector.tensor_copy(vaug[:, tkt, 0:HEAD_DIM], tp_ps[:, 0:HEAD_DIM])
                nc.vector.tensor_copy(vaug[:, tkt, HEAD_DIM + 1:2 * HEAD_DIM + 1],
                                      tp_ps[:, HEAD_DIM:2 * HEAD_DIM])

        def proj_units(w_, qrope_, xq0, xq1):
            return [
                lambda: project_q(2 * w_, qrope_, 0, xq0),
                lambda: project_kv(2 * w_, xq0),
                lambda: project_q(2 * w_ + 1, qrope_, 1, xq1),
                lambda: project_kv(2 * w_ + 1, xq1),
            ]

        def wo_units(pi, w0):
            """Split the post-exchange output projection into small units so
            they can be spread across the next window (avoids PE head-of-line
            stalls while the collective is still in flight)."""
            dma_units, units = [], []
            if ag_mode == 'a2a':
                # received: rows [src(8) x feat(512)], cols = my TSL tokens
                agt = agpool.tile([P, KH, 2, TSL], BF16, tag="ag")

                def u_dma():
                    for b in range(2):
                        ccv = cc_out[pi][b * NTP * QF:(b + 1) * NTP * QF, :] \
                            .rearrange("(t p) n -> p t n", p=P)
                        nc.scalar.dma_start(out=agt[:, :, b, :], in_=ccv)
                dma_units.append(u_dma)
                ov = out[:, :].rearrange("(t p) n -> p t n", p=P)
                MG = 4 if TSL <= 64 else 2
                for m0 in range(0, KH, MG):
                    def u_mm(m0=m0):
                        y_ps = ps_y.tile([P, MG, 2, TSL], F32, tag="y")
                        for mm in range(MG):
                            for t0 in range(KH):
                                nc.tensor.matmul(
                                    y_ps[:, mm, :, :],
                                    lhsT=wo_sb[:, t0,
                                               (m0 + mm) * P:(m0 + mm + 1) * P],
                                    rhs=agt[:, t0, :, :],
                                    start=(t0 == 0), stop=(t0 == KH - 1))
                        y_sb = small.tile([P, MG, 2, TSL], F32, tag="ysb")
                        nc.vector.tensor_copy(y_sb, y_ps)
                        for b in range(2):
                            nc.scalar.dma_start(
                                out=ov[:, b * KH + m0:b * KH + m0 + MG,
                                       w0 * 64:w0 * 64 + TSL],
                                in_=y_sb[:, :, b, :])
                    units.append(u_mm)
                return dma_units, units
            ccv = cc_out[pi][:, :].rearrange("(t p) n -> p t n", p=P)
            for sw in range(GSZ):
                ssl = slice(sw * TCA, (sw + 1) * TCA)
                osl = slice((w0 + sw) * TCA, (w0 + sw + 1) * TCA)
                ag = []

                def u_dma(ssl=ssl, ag=ag):
                    for k in range(NKW):
                        ag_t = agpool.tile([P, TCA], BF16, tag="ag")
                        nc.scalar.dma_start(out=ag_t, in_=ccv[:, k, ssl])
                        ag.append(ag_t)
                dma_units.append(u_dma)
                for m in range(4):
                    def u_mm(m=m, osl=osl, ag=ag):
                        y_ps = ps_y.tile([P, TCA], F32, tag="y")
                        for k in range(NKW):
                            nc.tensor.matmul(y_ps,
                                             lhsT=wo_sb[:, k, m * P:(m + 1) * P],
                                             rhs=ag[k], start=(k == 0),
                                             stop=(k == NKW - 1))
                        y_sb = small.tile([P, TCA], F32, tag="ysb")
                        nc.vector.tensor_copy(y_sb, y_ps)
                        nc.scalar.dma_start(out=out[m * P:(m + 1) * P, osl],
                                          in_=y_sb)
                    units.append(u_mm)
            return dma_units, units

        ready, aging, dmaq = [], [], []
        # window 0 projections run inline (nothing to interleave with)
        qrope = []
        for _ in range(4):
            qr_t = qrpool.tile([P, TCA], BF16, tag="qrope")
            qrope.append(qr_t)
        for u in proj_units(0, qrope, xc0, load_chunk(1)):
            u()
        for rep in range(repeat):
            for w in range(NW):
                ready = ready + aging
                aging = []
                wsl = slice(w * TCA, (w + 1) * TCA)
                last_win = (w + 1 == NW and rep + 1 == repeat)
                nw_ = (w + 1) % NW
                if not interleave and (w > 0 or rep > 0):
                    qrope = []
                    for _ in range(4):
                        qr_t = qrpool.tile([P, TCA], BF16, tag="qrope")
                        qrope.append(qr_t)
                    for u in proj_units(w, qrope, *xq):
                        u()
                if not last_win and interleave:
                    qrope_next = []
                    for _ in range(4):
                        qr_t = qrpool.tile([P, TCA], BF16, tag="qrope")
                        qrope_next.append(qr_t)
                projq = []

                # ---- attention window ----
                pi = rep * (NW // GSZ) + w // GSZ
                wl = w % GSZ
                if ag_mode == 'a2a':
                    ccinv = cc_in[pi][:, :].rearrange("(c h p) n -> p c h n",
                                                      c=NCORES, h=4)

                def emit_ccin(m):
                    if ag_mode == 'a2a':
                        atv = at_tiles[m][:, :].rearrange("p (c n) -> p c n",
                                                          c=NCORES)
                        nc.sync.dma_start(
                            out=ccinv[:, :, m, wl * 64:(wl + 1) * 64], in_=atv)
                    else:
                        psl = slice(wl * TCA, (wl + 1) * TCA)
                        nc.sync.dma_start(out=cc_in[pi][m * P:(m + 1) * P, psl],
                                          in_=at_tiles[m])

                n_tk = (w + 1) * WTK
                at_tiles = []
                for _ in range(4):
                    at_t = atpool.tile([P, TCA], BF16, tag="attnT")
                    at_tiles.append(at_t)
                for h in range(NHL):
                    g = h // (NHL // NKVL)
                    par = h % 2
                    base = par * HEAD_DIM
                    ksrc = KA if g == 0 else KB
                    qt = qrope[h // 2]
                    lsl = slice(base, base + HEAD_DIM)

                    pv_ps = ps_pv.tile([HEAD_DIM + 1, TCA], F32, tag="pv")
                    for i in range(n_tk):
                        o = i - w * WTK
                        lo = max(o, 0) * P
                        s_ps = ps_s.tile([P, TCA], F32, tag="s")
                        nc.tensor.matmul(
                            s_ps[:, lo:],
                            lhsT=ksrc[lsl, i * P:(i + 1) * P],
                            rhs=qt[lsl, lo:],
                            start=True, stop=True)
                        p_sb = ppool.tile([P, TCA], BF16, tag="p")
                        nc.scalar.activation(out=p_sb[:, lo:], in_=s_ps[:, lo:],
                                             func=mybir.ActivationFunctionType.Exp,
                                             scale=float(SCALE))
                        if o >= 0:
                            nc.gpsimd.tensor_tensor(out=p_sb[:, lo:lo + P],
                                                    in0=p_sb[:, lo:lo + P],
                                                    in1=msk_sb,
                                                    op=mybir.AluOpType.mult)
                        vsl = slice(g * (HEAD_DIM + 1), (g + 1) * (HEAD_DIM + 1))
                        nc.tensor.matmul(pv_ps[:, lo:], lhsT=vaug[:, i, vsl],
                                         rhs=p_sb[:, lo:],
                                         start=(i == 0), stop=(i == n_tk - 1))

                    rec = small.tile([1, TCA], F32R, tag="recip")
                    with nc.allow_low_precision(reason="f32r softmax denom"):
                        nc.vector.reciprocal(rec, pv_ps[HEAD_DIM:HEAD_DIM + 1, :])
                    rep_ps = ps_misc.tile([HEAD_DIM, TCA], F32, tag="misc")
                    nc.tensor.matmul(rep_ps, lhsT=ones_sb, rhs=rec,
                                     start=True, stop=True)
                    rep_sb = small.tile([HEAD_DIM, TCA], F32, tag="rep")
                    nc.vector.tensor_copy(rep_sb, rep_ps)
                    nc.vector.tensor_tensor(
                        out=at_tiles[h // 2][base:base + HEAD_DIM, :],
                        in0=pv_ps[0:HEAD_DIM, :], in1=rep_sb,
                        op=mybir.AluOpType.mult)

                    if ccin_spread and h % 2 == 1:
                        emit_ccin(h // 2)
                    if h == 1 and not last_win:
                        nsl = slice(nw_ * TCA, (nw_ + 1) * TCA)
                        nslot = (w + 1) % 2
                        nc.sync.dma_start(out=cos_sb[:, nslot, :],
                                          in_=cosT[:, nsl])
                        nc.sync.dma_start(out=sin_sb[:, nslot, :],
                                          in_=sinTs[:, nsl])
                        xq = (load_chunk(2 * nw_), load_chunk(2 * nw_ + 1))
                    if h == 2:
                        while dmaq:
                            dmaq.pop(0)()
                    if h == 3 and not last_win and interleave:
                        projq = proj_units(nw_, qrope_next, *xq)
                    if ready and h >= 3:
                        take = max(1, -(-len(ready) // (NHL - h)))
                        for _ in range(take):
                            if ready:
                                ready.pop(0)()
                    if projq and h >= 4:
                        take = max(1, -(-len(projq) // (NHL - h)))
                        for _ in range(take):
                            if projq:
                                projq.pop(0)()

                # ---- exchange attn^T across cores ----
                if not ccin_spread:
                    for m in range(4):
                        emit_ccin(m)
                if w % GSZ == GSZ - 1:
                    nc.gpsimd.collective_compute(
                        "AllToAll" if ag_mode == 'a2a' else "AllGather",
                        mybir.AluOpType.bypass,
                        replica_groups=groups,
                        ins=[cc_in[pi][:, :]],
                        outs=[cc_out[pi][:, :]],
                    )
                    du, mu = wo_units(pi, w - GSZ + 1)
                    dmaq += du
                    aging += mu
                while projq:
                    projq.pop(0)()
                if not last_win and interleave:
                    qrope = qrope_next

                if rep == 0 and w == 0:
                    for k in range(NKW):
                        nc.sync.dma_start(out=wo_sb[:, k, :], in_=wov[:, k, :])

        for u in dmaq + ready + aging:
            u()

    nc.compile()
    return nc


_NC_CACHE = {}


def _get_nc(T):
    if T not in _NC_CACHE:
        _NC_CACHE[T] = build_kernel(T, **KCFG)
    return _NC_CACHE[T]


def _perm64():
    """Per-head permutation: interleaved (even,odd) -> [r(32) | i(32)]."""
    p = np.empty(HEAD_DIM, dtype=np.int64)
    p[:32] = np.arange(0, HEAD_DIM, 2)
    p[32:] = np.arange(1, HEAD_DIM, 2)
    return p


def make_inputs(x, freqs_cis, wq, wk, wv, wo, T, ag_mode='full8'):
    """Build the 8 per-core input maps (host-side sharding + layout prep)."""
    perm = _perm64()
    f32 = np.float32

    cos = np.asarray(freqs_cis[:T, :, 0], dtype=f32)   # [T, 32]
    sin = np.asarray(freqs_cis[:T, :, 1], dtype=f32)
    cosT = np.tile(cos.T, (4, 1)).astype(ml_dtypes.bfloat16)                       # [128, T]
    sinTs = np.tile(np.vstack([-sin.T, sin.T]), (2, 1)).astype(ml_dtypes.bfloat16)

    J = np.zeros((HEAD_DIM, HEAD_DIM), dtype=f32)
    J[np.arange(32), np.arange(32) + 32] = 1.0
    J[np.arange(32) + 32, np.arange(32)] = 1.0
    swp = np.zeros((P, P), dtype=ml_dtypes.bfloat16)
    swp[:HEAD_DIM, :HEAD_DIM] = J
    swp[HEAD_DIM:, HEAD_DIM:] = J

    # single causal triangle mask [128, 128]: msk[p, q] = (q >= p)
    q_idx = np.arange(P)
    p_idx = np.arange(P)[:, None]
    msk = (q_idx[None, :] >= p_idx).astype(ml_dtypes.bfloat16)

    def permute_heads(w, n_heads):
        wh = np.asarray(w, f32).reshape(n_heads, HEAD_DIM, HIDDEN)
        return wh[:, perm, :].reshape(n_heads * HEAD_DIM, HIDDEN)

    wq_p = permute_heads(wq, N_HEADS)
    wk_p = permute_heads(wk, N_KV_HEADS)
    wv_n = np.asarray(wv, f32)
    wo_n = np.asarray(wo, f32)

    in_maps = []
    for core in range(NCORES):
        b, j = divmod(core, NTP)
        xTc = np.ascontiguousarray(np.asarray(x[b, :T], f32).T).astype(ml_dtypes.bfloat16)
        wqTc = np.ascontiguousarray(wq_p[j * QF:(j + 1) * QF].T).astype(ml_dtypes.bfloat16)
        wkTc = np.ascontiguousarray(wk_p[j * KF:(j + 1) * KF].T).astype(ml_dtypes.bfloat16)
        wvTc = np.ascontiguousarray(wv_n[j * KF:(j + 1) * KF].T).astype(ml_dtypes.bfloat16)
        if ag_mode == 'a2a':
            woTc = wo_n.T.astype(ml_dtypes.bfloat16)     # full wo, all cores
        else:
            woTc = np.zeros((2 * HIDDEN, COLS), dtype=ml_dtypes.bfloat16)
            off = b * HIDDEN if ag_mode == 'full8' else 0
            woTc[off:off + HIDDEN] = wo_n[j * COLS:(j + 1) * COLS].T.astype(
                ml_dtypes.bfloat16)              # own-batch rows only
        in_maps.append({
            "xT": xTc, "wqT": wqTc, "wkT": wkTc, "wvT": wvTc, "woT": woTc,
            "cosT": cosT, "sinTs": sinTs, "swp": swp, "msk": msk,
        })
    return in_maps


def assemble(core_outs, T, ag_mode='full8'):
    out = np.empty((B_FULL, T, HIDDEN), dtype=np.float32)
    if ag_mode == 'a2a':
        NW = T // TCA
        for core in range(NCORES):
            o = core_outs[core]                    # [2*HIDDEN, NW*64]
            for b in range(B_FULL):
                ob = o[b * HIDDEN:(b + 1) * HIDDEN]
                for w in range(NW):
                    out[b, w * TCA + core * 64:w * TCA + (core + 1) * 64, :] = \
                        ob[:, w * 64:(w + 1) * 64].T
    else:
        for core in range(NCORES):
            b, j = divmod(core, NTP)
            out[b, :, j * COLS:(j + 1) * COLS] = core_outs[core].T
    return out


KCFG = dict(ag_mode='a2a', gsz=2, interleave=False, ccin_spread=False)


def kernel(x, freqs_cis, wq, wk, wv, wo):
    global LAST_EXEC_NS, LAST_RESULTS
    T = x.shape[1]
    nc = _get_nc(T)
    in_maps = make_inputs(x, freqs_cis, wq, wk, wv, wo, T,
                          ag_mode=KCFG['ag_mode'])
    trace = bool(int(os.environ.get("KERNEL_TRACE", "0")))
    res = run_bass_kernel_spmd(nc, in_maps, core_ids=list(range(NCORES)),
                               trace=trace)
    LAST_EXEC_NS = res.exec_time_ns
    LAST_RESULTS = res
    return assemble([res.results[c]["out"] for c in range(NCORES)], T,
                    ag_mode=KCFG['ag_mode'])

